# revision 22
# baseline (speedup 1.0000x reference)
"""VQ codebook encoding (nn_Encoding) Trainium2 Bass kernel.

Math (per batch b):
    xf = x[b].reshape(C, N).T                      # (N, C)
    logits[n,k] = scale_k * (||xf_n||^2 - 2 xf_n.cw_k + ||cw_k||^2)
    w = softmax(logits, axis=k)
    enc[k,:]  = sum_n w[n,k] * (xf_n - cw_k)

Device decomposition (data-parallel over batch, 2 batches/core on 8 cores).
The host ships two pre-tiled layouts of x (no on-device transposes needed)
plus the tiny per-pixel norm row:
    - xin8:  fp8-e4m3 [c,n] tiling, feeds mm1 (softmax weights are
             insensitive to x quantization; verified rel-err 2e-3)
    - xt16:  bf16 [n,c] tiling, feeds the aggregation matmul
    - x2:    bf16 [1, N] per batch, ||x_n||^2 (host-computed reduction)
ALL input DMAs are issued up front (the whole per-core input fits in SBUF)
so the PE runs dense and the HAM clock-gate stays at 2.4 GHz; a burst of
tiny warm-up matmuls engages it while the first loads land.
Per 512-pixel segment:
    - PE mm1: lgT[k,n] = sum_cc at8^T x8  +  sbcrow^T x2row (rank-1), where
      at8 = e4m3(-128*scale_k*cw) and sbcrow = bf16(64*scale) carry a 64x
      prescale (keeps at8 out of the fp8 subnormal floor); Exp un-scales.
    - ACT: numer = Exp(lgT/64 + bias) -> bf16 SBUF.
    - PE: numer chunks transposed via identity matmul -> nt [n,k] PSUM.
    - DVE: denominators = tensor_reduce(nt, axis=k); reciprocal;
      wt[:, s] = nt * rden (stride-0 broadcast) -> bf16, kept per batch.
    - PE mm2 (issued one segment behind so the DVE chain overlaps the next
      segment's mm1): 4-way COLUMN-TILED (tile_position=(0,32*nb)),
      accumulating into partition strips of one PSUM bank over all segments.
Per batch: wsum via 2 matmuls over the whole wt tile (negones stationary)
+ DVE free-dim reduce + 32x32 DVE transpose; strip-reduce enc with a
stacked-identity matmul; enc += wsum*cw; DMA out.
"""
import os
import numpy as np

B, C, N, K = 16, 512, 4096, 32
NCORES = 8
BPC = B // NCORES          # batches per core
SEG = 512                  # n per segment
NSEG = N // SEG
CC = C // 128              # c chunks
NB = SEG // 128            # n chunks per segment
PRE = 64.0                 # fp8 prescale for mm1 operands
LGRP = 2                   # segments per DMA load group
NGRP = NSEG // LGRP
NWARM = 9

_CACHE = {}


def _patch_tile_drain(tile, mybir, ScopedClock):
    """This walrus build rejects any instruction carrying >1 sync wait.
    Split extra waits onto single-wait NoOps on the same engine."""
    if getattr(tile.TileContext, "_multiwait_patched", False):
        return
    tile.TileContext._multiwait_patched = True

    _orig_add = tile.TileContext._add_instruction

    def _split_add(self, inst):
        si = inst.sync_info
        if si is not None and si.on_wait and len(si.on_wait) > 1:
            waits = list(si.on_wait)
            for w in waits[:-1]:
                nop = mybir.InstNoOp(name=f"waitnop-{self.nc.next_id()}", ins=[], outs=[])
                nop.engine = inst.engine
                nop.sync_info = mybir.SyncInfo(on_wait=[w], on_update=[])
                _orig_add(self, nop)
            si.on_wait = [waits[-1]]
            inst.sync_info = si
        _orig_add(self, inst)

    tile.TileContext._add_instruction = _split_add

    def _patched_drain(self, tick_clock, wait_clock):
        nc = self.nc
        probe = nc.sync.drain()
        wait_clock.add_sem_waits(probe.ins, ScopedClock({None: tick_clock.global_clock}))
        raw = probe.ins
        waits = list(raw.sync_info.on_wait) if raw.sync_info and raw.sync_info.on_wait else []
        if raw.sync_info is not None:
            raw.sync_info.on_wait = []
        for w in waits:
            wi = nc.sync.nop()
            wi.ins.sync_info = mybir.SyncInfo(on_wait=[w], on_update=[])
        nc.all_engine_barrier()
        assert self.sems is not None
        popped = nc._tile_sem_poison_stack.pop()
        assert popped is self._sem_poison
        nc.clear_and_free_semaphores(list(self.sems.allocated().values()))
        nc.all_engine_barrier()

    tile.TileContext._drain_and_barrier = _patched_drain


def _build():
    import concourse.bass as bass
    import concourse.tile as tile
    from concourse import mybir
    from concourse.vector_clock import ScopedClock

    _patch_tile_drain(tile, mybir, ScopedClock)

    F32 = mybir.dt.float32
    F32R = mybir.dt.float32r
    BF16 = mybir.dt.bfloat16
    FP8 = mybir.dt.float8e4
    Alu = mybir.AluOpType
    Act = mybir.ActivationFunctionType
    Axis = mybir.AxisListType

    nc = bass.Bass("TRN2", target_bir_lowering=False, debug=False, num_devices=NCORES)
    xin_ext = nc.dram_tensor(
        "xin", [BPC, NGRP, 128, LGRP * CC * SEG], FP8, kind="ExternalInput").ap()
    xt_ext = nc.dram_tensor(
        "xt", [BPC, NGRP, 128, LGRP * NB * C], BF16, kind="ExternalInput").ap()
    x2_ext = nc.dram_tensor("x2", [1, BPC * N], BF16, kind="ExternalInput").ap()
    at_ext = nc.dram_tensor("at", [128, CC, K], FP8, kind="ExternalInput").ap()
    sbc_ext = nc.dram_tensor("sbcrow", [1, K], BF16, kind="ExternalInput").ap()
    bias_ext = nc.dram_tensor("bias", [K, 1], F32, kind="ExternalInput").ap()
    cw_ext = nc.dram_tensor("cw", [K, C], F32, kind="ExternalInput").ap()
    id32_ext = nc.dram_tensor("ident32", [K, K], BF16, kind="ExternalInput").ap()
    id4_ext = nc.dram_tensor("ident4", [128, K], F32, kind="ExternalInput").ap()
    enc_ext = nc.dram_tensor("enc", [BPC, K, C], F32, kind="ExternalOutput").ap()

    with tile.TileContext(nc) as tc:
        with (
            tc.tile_pool(name="singles", bufs=1) as singles,
            tc.tile_pool(name="xin", bufs=BPC * NGRP) as xin,
            tc.tile_pool(name="xts", bufs=BPC * NGRP) as xts,
            tc.tile_pool(name="wts", bufs=2) as wts,
            tc.tile_pool(name="small", bufs=2) as small,
            tc.tile_pool(name="outp", bufs=2) as outp,
            tc.tile_pool(name="ps_lg", bufs=2, space="PSUM") as ps_lg,
            tc.tile_pool(name="ps_nt", bufs=2, space="PSUM") as ps_nt,
            tc.tile_pool(name="ps_enc", bufs=2, space="PSUM") as ps_enc,
            tc.tile_pool(name="ps_ws", bufs=1, space="PSUM") as ps_ws,
            tc.tile_pool(name="ps_red", bufs=1, space="PSUM") as ps_red,
        ):
            # ---- PE warm-up: engage HAM while the first loads land.
            # Full-width 512-col streams keep the PE ~fully busy so the
            # HAM activity window actually flips to 2.4 GHz. ----
            warm_sb = singles.tile([128, K], BF16)
            nc.vector.memset(warm_sb, 0.0)
            warm_mv = singles.tile([128, SEG], BF16)
            nc.vector.memset(warm_mv, 0.0)
            for _ in range(NWARM):
                warm_ps = ps_red.tile([K, SEG], F32, tag="encR")
                nc.tensor.matmul(warm_ps, warm_sb, warm_mv,
                                 start=True, stop=True)

            at_sb = singles.tile([128, CC, K], FP8)
            nc.gpsimd.dma_start(out=at_sb, in_=at_ext)
            sbc_sb = singles.tile([1, K], BF16)
            nc.gpsimd.dma_start(out=sbc_sb, in_=sbc_ext)
            x2_sb = singles.tile([1, BPC * N], BF16)
            nc.gpsimd.dma_start(out=x2_sb, in_=x2_ext)
            bias_sb = singles.tile([K, 1], F32)
            nc.gpsimd.dma_start(out=bias_sb, in_=bias_ext)
            id32_sb = singles.tile([K, K], BF16)
            nc.gpsimd.dma_start(out=id32_sb, in_=id32_ext)
            cw_sb = singles.tile([K, C], F32)
            nc.gpsimd.dma_start(out=cw_sb, in_=cw_ext)
            id4_f = singles.tile([128, K], F32)
            nc.gpsimd.dma_start(out=id4_f, in_=id4_ext)
            id4_sb = singles.tile([128, K], F32R)
            nc.vector.tensor_copy(out=id4_sb, in_=id4_f)
            negones = singles.tile([128, 8], BF16)
            nc.vector.memset(negones, -1.0)

            # ---- all input DMAs issued up front (fits in SBUF) ----
            x8g = {}
            xtg = {}
            for b in range(BPC):
                for g in range(NGRP):
                    x8g[b, g] = xin.tile([128, LGRP, CC, SEG], FP8, tag="x8",
                                         name=f"x8g_{b}_{g}")
                    nc.sync.dma_start(
                        out=x8g[b, g],
                        in_=xin_ext[b, g].rearrange(
                            "p (l cc n) -> p l cc n", l=LGRP, cc=CC))
                    xtg[b, g] = xts.tile([128, LGRP, NB, C], BF16, tag="xt",
                                         name=f"xtg_{b}_{g}")
                    nc.sync.dma_start(
                        out=xtg[b, g],
                        in_=xt_ext[b, g].rearrange(
                            "p (l nb c) -> p l nb c", l=LGRP, nb=NB))

            for b in range(BPC):
                enc4_ps = ps_enc.tile([128, C], F32, tag="enc4")
                wtall = wts.tile([128, NSEG, NB, K], BF16, tag="wt")
                numers = {}

                def _softmax_tail(s):
                    # nt transposes + normalize for segment s (issued at s+1)
                    nt_ps = ps_nt.tile([128, NB, K], BF16, tag="nt",
                                       name=f"nt_{b}_{s}")
                    numer_s = numers.pop(s)
                    for nb in range(NB):
                        nc.tensor.transpose(
                            nt_ps[:, nb, :],
                            numer_s[:, nb * 128:(nb + 1) * 128],
                            id32_sb)
                    dcols = small.tile([128, NB], F32, tag="dc",
                                       name=f"dc_{b}_{s}")
                    nc.vector.tensor_reduce(
                        out=dcols, in_=nt_ps, axis=Axis.X, op=Alu.add)
                    rden = small.tile([128, NB], F32, tag="rd",
                                      name=f"rd_{b}_{s}")
                    nc.vector.reciprocal(rden, dcols)
                    nc.vector.tensor_mul(
                        out=wtall[:, s], in0=nt_ps,
                        in1=rden.broadcast_to([128, NB, K]))

                for s in range(NSEG):
                    g, o = divmod(s, LGRP)
                    x8 = x8g[b, g][:, o]
                    # ---- mm1: lgT [K, 512] = 64*scale*(x2 - 2 xc) ----
                    lg_ps = ps_lg.tile([K, SEG], F32, tag="lg")
                    for cc in range(CC):
                        nc.tensor.matmul(lg_ps, at_sb[:, cc, :], x8[:, cc, :],
                                         start=(cc == 0), stop=False)
                    n0 = (b * NSEG + s) * SEG
                    nc.tensor.matmul(lg_ps, sbc_sb, x2_sb[:, n0:n0 + SEG],
                                     start=False, stop=True)
                    if b == 0 and 1 <= s <= 6:
                        # filler warm matmuls: the early segments are
                        # DMA-paced; keep the HAM activity window busy so
                        # the PE clock stays at 2.4 GHz through the stalls
                        for _ in range(2):
                            warm_ps = ps_red.tile([K, SEG], F32, tag="encR",
                                                  name=f"warmf_{s}")
                            nc.tensor.matmul(warm_ps, warm_sb, warm_mv,
                                             start=True, stop=True)
                    # ---- softmax numerator ----
                    numer = small.tile([K, SEG], BF16, tag="numer")
                    nc.scalar.activation(out=numer, in_=lg_ps, func=Act.Exp,
                                         bias=bias_sb, scale=1.0 / PRE)
                    numers[s] = numer
                    # ---- pipelined tails: nt(s-1), mm2(s-2) ----
                    if s >= 1:
                        _softmax_tail(s - 1)
                    if s >= 2:
                        _emit_mm2(nc, enc4_ps, wtall, xtg, b, s - 2,
                                  s - 2 == 0, False)
                _softmax_tail(NSEG - 1)
                _emit_mm2(nc, enc4_ps, wtall, xtg, b, NSEG - 2, False, False)
                _emit_mm2(nc, enc4_ps, wtall, xtg, b, NSEG - 1, False, True)
                # ---- batch epilogue ----
                # wsum over all segments: 2 accumulating matmuls + reduce
                ws_ps = ps_ws.tile([1, NSEG // 2 * NB * K], F32, tag="ws")
                for h in range(2):
                    nc.tensor.matmul(
                        ws_ps, negones[:, 0:1],
                        wtall[:, h * (NSEG // 2):(h + 1) * (NSEG // 2)],
                        start=(h == 0), stop=(h == 1))
                wsrow = outp.tile([K, K], F32, tag="wsrow")
                nc.vector.tensor_reduce(
                    out=wsrow[0:1, :],
                    in_=ws_ps.rearrange("p (g k) -> p k g", k=K),
                    axis=Axis.X, op=Alu.add)
                wscol = outp.tile([K, K], F32, tag="wscol")
                nc.vector.transpose(out=wscol, in_=wsrow)
                # strip-reduce enc + cw fixup
                enc4_sb = outp.tile([128, C], F32R, tag="enc4_sb")
                nc.scalar.copy(out=enc4_sb, in_=enc4_ps)
                encR_ps = ps_red.tile([K, C], F32, tag="encR")
                nc.tensor.matmul(encR_ps, id4_sb, enc4_sb, start=True, stop=True)
                enc_sb = outp.tile([K, C], F32, tag="enc_out")
                nc.vector.scalar_tensor_tensor(
                    out=enc_sb, in0=cw_sb, scalar=wscol[:, 0:1], in1=encR_ps,
                    op0=Alu.mult, op1=Alu.add)
                nc.sync.dma_start(out=enc_ext[b], in_=enc_sb)

    return nc


def _emit_mm2(nc, enc4_ps, wtall, xtg, b, s, first, last):
    g, o = divmod(s, LGRP)
    for nb in range(NB):
        nc.tensor.matmul(
            enc4_ps[32 * nb:32 * (nb + 1), :],
            wtall[:, s, nb, :], xtg[b, g][:, o, nb, :],
            start=first, stop=last,
            tile_position=(0, 32 * nb),
            skip_group_check=True)


def kernel(x, codewords, scale):
    from concourse.bass_utils import run_bass_kernel_spmd
    import ml_dtypes

    x = np.ascontiguousarray(x, dtype=np.float32)
    codewords = np.ascontiguousarray(codewords, dtype=np.float32)
    scale = np.ascontiguousarray(scale, dtype=np.float32)

    if "nc" not in _CACHE:
        _CACHE["nc"] = _build()
    nc = _CACHE["nc"]

    # host-side prep: two tiled layouts of x + per-pixel norms
    xr = x.reshape(B, C, N)
    # xin8[b, g, p, (l, cc, n)] = x[b, cc*128+p, (g*LGRP+l)*SEG+n]
    xin8 = np.ascontiguousarray(
        xr.reshape(B, CC, 128, NGRP, LGRP, SEG).transpose(0, 3, 2, 4, 1, 5)
        .reshape(B, NGRP, 128, LGRP * CC * SEG)).astype(ml_dtypes.float8_e4m3)
    # xt16[b, g, p, (l, nb, c)] = x[b, c, (g*LGRP+l)*SEG + nb*128 + p]
    xt16 = np.ascontiguousarray(
        xr.transpose(0, 2, 1).reshape(B, NGRP, LGRP, NB, 128, C)
        .transpose(0, 1, 4, 2, 3, 5)
        .reshape(B, NGRP, 128, LGRP * NB * C)).astype(ml_dtypes.bfloat16)
    x2 = np.einsum('bcn,bcn->bn', xr, xr).astype(ml_dtypes.bfloat16)  # [B, N]

    at = (-2.0 * PRE * scale[:, None] * codewords).T.copy()     # [C, K]
    at8 = at.reshape(CC, 128, K).transpose(1, 0, 2).astype(ml_dtypes.float8_e4m3)
    at8 = np.ascontiguousarray(at8)                             # [128, cc, K]
    sbcrow = (PRE * scale).reshape(1, K).astype(ml_dtypes.bfloat16)
    c2 = (codewords.astype(np.float64) ** 2).sum(1).astype(np.float32)
    bias = (scale * c2).reshape(K, 1).astype(np.float32)
    ident32 = np.eye(K, dtype=ml_dtypes.bfloat16)
    # stacked identity: id4[32j + i, k] = (i == k)
    id4 = np.tile(np.eye(K, dtype=np.float32), (NB, 1))         # [128, K]

    in_maps = []
    for i in range(NCORES):
        in_maps.append({
            "xin": np.ascontiguousarray(xin8[i * BPC:(i + 1) * BPC]),
            "xt": np.ascontiguousarray(xt16[i * BPC:(i + 1) * BPC]),
            "x2": np.ascontiguousarray(
                x2[i * BPC:(i + 1) * BPC].reshape(1, BPC * N)),
            "at": at8, "sbcrow": sbcrow, "bias": bias,
            "cw": codewords, "ident32": ident32, "ident4": id4,
        })
    tmpdir = os.environ.get("BASS_PROF_DIR") or None
    res = run_bass_kernel_spmd(nc, in_maps, list(range(NCORES)), tmpdir=tmpdir)
    _CACHE["last_results"] = res
    out = np.concatenate([res.results[i]["enc"] for i in range(NCORES)], axis=0)
    return out.astype(np.float32)
